# revision 23
# baseline (speedup 1.0000x reference)
"""Bilinear MoE-routing scores on 8 Trainium2 NeuronCores.

Problem: scores[n,k] = u[n,:] @ W_k @ v[n,:]; out[n] = sigmoid(scores[n, type_idx[n]]).
N=131072, D=256, K=8.

Sharding: rows grouped BY TYPE — core k gets exactly the rows with
type_idx == k, so each core runs one plain bilinear kernel against its own
W_k (8x less matmul work than data-parallel all-K). Host argsorts, pads each
group to a common n_pad, and scatters per-core results back to row order.

Precision: pure fp16 for u, v, AND W (validated offline on the exact
key=0 inputs: 6.0e-3 absmax output error vs the 2e-2 gate — fp32 PSUM
accumulation keeps products exact, only input rounding contributes).
This halves DMA bytes vs fp16-hi/lo + fp32-v: 1 KiB/row -> ~17 MB/core.

Device kernel (per core, SPMD):
  t[n,e] = sum_d uT[d,n] * W[d,e]   TensorE, fp16 x fp16 -> fp32 PSUM,
                                    2 matmuls per 128-row tile (~28 us)
  s[n]   = sum_e t[n,e] * v[n,e]    VectorE: one fused mul+cumsum custom op
                                    (MUL_CUMSUM_ANT, registered at import)
                                    per 8-tile PSUM supergroup; per-tile sums
                                    recovered as differences of the running
                                    sum at tile boundaries (ScalarE extracts
                                    the boundary column, VectorE diffs).
                                    ~37 us vs ~51 us for per-tile
                                    affine_mul_reduce (PSUM-source DVE ops run
                                    1x with a 120-cycle per-instruction
                                    penalty; the cumsum amortizes it 8x).
  out[n] = sigmoid(s[n])            ScalarE LUT

Measured (8 cores concurrent, paired-delta slope over 1024 on-device
iterations): ~47 us best / ~52-56 us under ambient HBM contention, vs
~52 us for a DMA-only ablation of the same traffic in the same window —
i.e. compute is fully hidden and the kernel sits at the HBM roofline
(16.9 MB/core; per-NC HBM limit is ~358 GB/s = 716 GB/s/stack / 2 NCs).
CHUNK=1536 with 6 DMA buffers edged out 1024/6 once the repeat loop was
unrolled (51.80 vs 52.06 med interleaved; pre-unroll, 1024/6 had beaten
2048/3 and 2048/6 — ambient load drifts minute-to-minute, so configs
must be compared interleaved in one process).  Sub-512-row leftover
chunks are folded into a neighboring mid chunk (a 128-row chunk means
256 B u-descriptor runs, below SDMA line rate).  u and v are packed
into ONE dram tensor laid out per chunk [u_h0 | u_h1 | v], so each
chunk is a single DMA (13/body instead of 26) with 4*ch contiguous
fp16 per partition, alternating between the two HWDGE rings per chunk:
-0.66 us interleaved (58.39 vs 59.05), consistent across quartiles —
same mechanism as the 1536-chunk win (fewer DMA instructions -> less
queue/semaphore pressure).  Packed-pool depth curve (in-flight DMAs):
5 -> 55.7, 6 -> 53.2, 8 -> 52.4 (optimum, adopted), 10 -> 52.8,
12 -> 52.7 us — packing halved the in-flight count, shifting the
optimum from the pre-pack 6+6 split pools to 8.  CHUNK=2048 in the
packed regime (16 KB descriptor runs/partition, 11 chunks/body) beat
1536 by a wide same-window margin (51.8 vs 58.9 med, whole
distribution shifted; pre-packing 2048 had LOST to 1536 — chunk-size
conclusions do not transfer across DMA-layout changes).  Removing the
fill/drain taper entirely is neutral-to-worse (59.3) — keep it.
Chunk DMAs alternate HWDGE rings via a global cross-body counter
(odd chunks/body would pin body boundaries to one ring). UNROLL=4 invocations per For_i
iteration amortizes the ~2.2 us all-engine loop-reset barrier and lets
consecutive invocations' fill/drain overlap (parity-duplicated score
buffers break the WAR chain): 61.7 -> 54.1 (UNROLL=4) -> 52.7 us
(UNROLL=8) interleaved A/B medians; the unrolled dma_only floor is
~49.2 us, so ~3.5 us of cross-engine intrusion remains.  Duplicating
w_sb (PE weight reads alternate two SBUF copies per row-tile, halving
per-bank read pressure against concurrent DMA writes) bought a further
~0.5 us in two independent interleaved A/Bs.  Deeper DMA pools
(UBUFS/VBUFS 10) regress +2 us: more in-flight DMA worsens queueing.
G=4 with a 4-deep PSUM pool is neutral.
Rejected on HW measurement: end-taper chunks < 512 B/descriptor (+2 us:
sub-line-rate DMA on the latency-critical final bytes), offloading 1/3
of the reduction to a ScalarE evict+accum path (+18 us: real ScalarE
activation throughput is far below the (172+FD)/1.2GHz model), per-tile
affine_mul_reduce on fp16 SBUF (custom DVE ops have no 2x uops — all
reduces on DVE run 1x, so the fused PSUM-source scan is already
optimal). Baseline (fp16-hi/lo u + fp32 v, per-tile affine_mul_reduce):
~119 us.
"""

import math

import numpy as np

P = 128  # SBUF partitions
D = 256  # hidden dim
N_CORES = 8
CHUNK = 2048  # rows per DMA chunk (multiple of 128)
UBUFS = 8  # deep prefetch rides through transient HBM-contention dips
VBUFS = 6
G = 8  # row-tiles per PSUM super-tile ([128, G*256] f32 = G/2 banks)
SCAN_MODE = True  # fused mul+cumsum custom DVE op vs per-tile affine_mul_reduce
STAGGER = False  # staggered semaphore reset in the benchmark repeat loop (crashes HW)
UNROLL = 8  # kernel invocations per For_i iteration in the repeat loop.
# The For_i reset is an all-engine barrier (~2.2 us measured via the
# 'empty' ablation) and the fill/drain tail (~4 us) cannot cross it, so
# back-to-back invocations in one body pipeline naturally (score/output
# buffers are parity-duplicated to break WAR serialization) and the
# barrier+tail cost is amortized over UNROLL invocations — the slope
# then measures the true amortized streaming cost per invocation.

_PROGRAM_CACHE: dict = {}
_SCAN_OP = None


def _get_scan_op():
    """Register (once) and return the MUL_CUMSUM_ANT custom DVE op:
    out[p, k] = sum_{j<=k} in0[p, j] * in1[p, j]  (fp32 internal state).

    One fused 1x-rate pass replaces the per-tile affine_mul_reduce calls;
    per-instruction overhead is amortized over G row-tiles."""
    global _SCAN_OP
    if _SCAN_OP is not None:
        return _SCAN_OP
    import concourse.dve_ops as dve_ops
    from concourse.dve_spec import Spec, Src0, Src1, AluOp, scan, lower
    from concourse.dve_uop import DveOpSpec

    for o in dve_ops.OPS:
        if o.name == "MUL_CUMSUM_ANT":
            _SCAN_OP = o
            return o

    def _ref(in0, in1, s0, s1, imm2):
        return np.cumsum(
            in0.astype(np.float32) * np.asarray(in1, dtype=np.float32),
            axis=-1,
            dtype=np.float32,
        )

    spec = Spec(body=scan(AluOp.ADD, Src0 * Src1), reference=_ref)
    shas = {}
    for ver in ("v3", "v4"):
        uops = lower(spec, ver=ver)
        shas[ver] = DveOpSpec(
            name="MUL_CUMSUM_ANT", opcode=0, uops=uops, rd1_en=True
        ).sha(ver)
    op = dve_ops.DveOp("MUL_CUMSUM_ANT", spec, subdim=False, uops_sha=shas)
    dve_ops.OPS.append(op)
    dve_ops.CUSTOM_DVE_SPECS[op.name] = spec
    dve_ops._SUB_OPCODE_FOR_NAME[op.name] = (
        dve_ops._CUSTOM_DVE_ROW_BASE + len(dve_ops.OPS) - 1
    )
    _SCAN_OP = op
    return op


def _chunk_sizes(n_pad: int):
    """Small chunks at both ends for pipeline fill/drain, CHUNK in the middle.
    All sizes are multiples of 128; sum == n_pad."""
    rem = n_pad
    up = []
    for s in (512, 1024):
        if rem >= s + 1536 + CHUNK:
            up.append(s)
            rem -= s
    down = []
    for s in (1024, 512):
        if rem >= s + 512:
            down.append(s)
            rem -= s
    n_mid = rem // CHUNK
    leftover = rem - n_mid * CHUNK
    mids = [CHUNK] * n_mid
    if leftover and leftover < 512 and mids:
        # a sub-512-row chunk means sub-512B u descriptor runs (below SDMA
        # line rate) plus an extra DMA instruction — fold it into one mid
        # chunk instead
        mids[-1] += leftover
        leftover = 0
    sizes = up + mids + ([leftover] if leftover else []) + down
    assert sum(sizes) == n_pad and all(s % P == 0 for s in sizes)
    return sizes


def _build_program(n_pad: int, repeat: int = 1, mode: str = "full"):
    """Build + compile the SPMD Bass program for n_pad rows per core.

    repeat > 1 wraps the body in an on-device loop (benchmarking only).
    mode: 'full' | 'no_dve' | 'no_pe' | 'dma_only' (ablation benches)."""
    import contextlib

    import concourse.bass as bass  # noqa: F401
    import concourse.mybir as mybir
    import concourse.tile as tile
    from concourse import bacc

    do_pe = mode in ("full", "no_dve")
    do_dve = mode in ("full", "no_pe")
    do_dma = mode != "empty"

    f32 = mybir.dt.float32
    f16 = mybir.dt.float16
    n_tiles = n_pad // P
    assert n_pad % P == 0
    scan_op = _get_scan_op() if SCAN_MODE else None
    # number of supergroups (for the boundary-column buffer)
    n_groups = sum(
        len(range(0, ch // P, G)) for ch in _chunk_sizes(n_pad)
    )

    nc = bacc.Bacc(
        "TRN2", target_bir_lowering=False, debug=False, num_devices=N_CORES
    )
    # packed per-chunk: for chunk at rows [c0, c0+ch): u half0 (ch), u
    # half1 (ch), then v (2*ch) — one DMA per chunk, 4*ch contiguous fp16
    # per partition, issued on alternating HWDGE rings.
    pk = nc.dram_tensor("pk", [P, 4 * n_pad], f16, kind="ExternalInput").ap()
    # w pre-permuted: w_p[p, h, e] = W[h*128+p, e]
    w_p = nc.dram_tensor("w_p", [P, 2, D], f16, kind="ExternalInput").ap()
    out = nc.dram_tensor("out", [n_pad], f32, kind="ExternalOutput").ap()

    with tile.TileContext(nc) as tc:
        with (
            tc.tile_pool(name="singles", bufs=1) as singles,
            tc.tile_pool(name="pkpool", bufs=UBUFS) as pkpool,
            tc.tile_pool(name="ppool", bufs=max(1, 16 // G), space="PSUM") as ppool,
            tc.tile_pool(name="psingles", bufs=1, space="PSUM") as psingles,
            tc.tile_pool(name="spool", bufs=2) as spool,
        ):
            unroll = UNROLL if repeat > 1 else 1
            assert repeat == 1 or repeat % unroll == 0
            rep_ctx = (
                tc.For_i(
                    0,
                    repeat // unroll,
                    1,
                    hint_engines=(
                        mybir.EngineType.PE,
                        mybir.EngineType.DVE,
                        mybir.EngineType.Activation,
                    ),
                    staggered_reset=STAGGER,
                )
                if repeat > 1
                else contextlib.nullcontext()
            )

            # parity-duplicated so consecutive unrolled invocations overlap
            # without WAR serialization on the score/output buffers
            npar = min(unroll, 2)
            # s_buf[p, t] = score of padded row t*128+p
            s_bufs = [
                singles.tile([P, n_tiles], f32, tag=f"s{b}", name=f"s_buf{b}")
                for b in range(npar)
            ]
            sig_bufs = [
                singles.tile([P, n_tiles], f32, tag=f"sig{b}", name=f"sig_buf{b}")
                for b in range(npar)
            ]
            w_sbs = [
                singles.tile([P, 2, D], f16, tag=f"w{i}", name=f"w_sb{i}")
                for i in range(2)
            ]
            for _w in w_sbs:
                nc.scalar.dma_start(out=_w, in_=w_p)
            cum_bufs = []
            if SCAN_MODE:
                # cum[p, 0] = 0 permanently; the scan writes running sums of
                # the supergroup's products into cum[p, 1:1+g*D], so the sum
                # of tile j is cum[(j+1)*D] - cum[j*D] — one strided VectorE
                # subtract straight off the scan output, no ScalarE
                # boundary-copy hop.
                for b in range(2):
                    e = singles.tile([P, G * D + 1], f32, tag=f"cum{b}", name=f"cum{b}")
                    nc.vector.memset(e[:, 0:1], 0.0)
                    cum_bufs.append(e)
            static_ps = None
            if do_dve and not do_pe:
                # ablation: pre-zeroed PSUM tiles so the DVE reads allocated
                # data without any PE work inside the loop
                sps0 = psingles.tile([P, G, D], f32, tag="sps0")
                sps1 = psingles.tile([P, G, D], f32, tag="sps1")
                static_ps = [sps0, sps1]
                for sp in static_ps:
                    nc.vector.memset(sp, 0.0)

            out_pt = out.rearrange("(p t) -> p t", p=P)
            gci = [0]  # global chunk counter: strict HWDGE ring alternation
            # across body boundaries (odd chunks/body would otherwise put two
            # consecutive chunks on the same ring at every boundary)

            def emit_body(b):
                s_buf = s_bufs[b]
                sig_buf = sig_bufs[b]
                if not do_dma:
                    # empty-loop overhead probe: one tiny DVE op per body
                    nc.vector.memset(s_buf[:, 0:1], 0.0)
                c0 = 0
                gi = 0
                for ci, ch in enumerate(_chunk_sizes(n_pad) if do_dma else []):
                    cht = ch // P
                    t0 = c0 // P
                    pkt = pkpool.tile([P, 4 * ch], f16, tag="pk")
                    eng = nc.sync if gci[0] % 2 == 0 else nc.scalar
                    gci[0] += 1
                    eng.dma_start(out=pkt, in_=pk[:, 4 * c0 : 4 * c0 + 4 * ch])

                    for st in range(0, cht, G):
                        g = min(G, cht - st)
                        if do_pe:
                            ps = ppool.tile([P, g, D], f32, tag="ps")
                            for j in range(g):
                                w_sb = w_sbs[(st + j) % 2]
                                u0 = pkt[:, (st + j) * P : (st + j + 1) * P]
                                u1 = pkt[:, ch + (st + j) * P : ch + (st + j + 1) * P]
                                nc.tensor.matmul(
                                    ps[:, j, :], u0, w_sb[:, 0, :],
                                    start=True, stop=False,
                                )
                                nc.tensor.matmul(
                                    ps[:, j, :], u1, w_sb[:, 1, :],
                                    start=False, stop=True,
                                )
                        elif do_dve:
                            ps = static_ps[gi % 2][:, :g, :]
                        gt = t0 + st
                        if do_dve and SCAN_MODE:
                            cum = cum_bufs[gi % 2]
                            nc.vector._custom_dve(
                                scan_op,
                                out=cum[:, 1 : 1 + g * D],
                                in0=ps.rearrange("p g d -> p (g d)"),
                                in1=pkt[
                                    :,
                                    2 * ch + st * D : 2 * ch + (st + g) * D,
                                ],
                            )
                            hi = cum[:, 1 : 1 + g * D].rearrange(
                                "p (g d) -> p g d", d=D
                            )[:, :, D - 1]
                            lo = cum[:, 0 : g * D].rearrange(
                                "p (g d) -> p g d", d=D
                            )[:, :, 0]
                            nc.vector.tensor_tensor(
                                out=s_buf[:, gt : gt + g],
                                in0=hi,
                                in1=lo,
                                op=mybir.AluOpType.subtract,
                            )
                        elif do_dve:
                            scr = spool.tile([P, g, D], f32, tag="scr")
                            for j in range(g):
                                nc.vector.affine_mul_reduce(
                                    out=scr[:, j, :],
                                    accum_out=s_buf[:, gt + j : gt + j + 1],
                                    in0=ps[:, j, :],
                                    in1=pkt[:, 2 * ch + (st + j) * D : 2 * ch + (st + j + 1) * D],
                                    scale=1.0,
                                    bias=0.0,
                                )
                        gi += 1
                    c0 += ch

                # incremental sigmoid + output drain
                if do_dve:
                    n_blk = 4
                    bnd = [round(i * n_tiles / n_blk) for i in range(n_blk + 1)]
                    for b0, b1 in zip(bnd[:-1], bnd[1:]):
                        if b1 > b0:
                            nc.scalar.activation(
                                out=sig_buf[:, b0:b1],
                                in_=s_buf[:, b0:b1],
                                func=mybir.ActivationFunctionType.Sigmoid,
                            )
                            nc.sync.dma_start(
                                out=out_pt[:, b0:b1], in_=sig_buf[:, b0:b1]
                            )

            with rep_ctx:
                for b in range(unroll):
                    emit_body(b % npar)

    nc.compile()
    return nc


def _get_program(n_pad: int):
    if n_pad not in _PROGRAM_CACHE:
        _PROGRAM_CACHE[n_pad] = _build_program(n_pad)
    return _PROGRAM_CACHE[n_pad]


def _prep(u, v, weights, type_idx):
    """Group rows by type, pad, cast fp16, build per-core input maps."""
    u = np.ascontiguousarray(np.asarray(u, dtype=np.float32))
    v = np.ascontiguousarray(np.asarray(v, dtype=np.float32))
    weights = np.ascontiguousarray(np.asarray(weights, dtype=np.float32))
    ti = np.asarray(type_idx).astype(np.int64).ravel()

    n, d = u.shape
    k = weights.shape[0]
    assert d == D and k == N_CORES

    order = np.argsort(ti, kind="stable")
    counts = np.bincount(ti, minlength=k)
    offsets = np.concatenate(([0], np.cumsum(counts)))
    n_pad = max(P, int(math.ceil(counts.max() / P)) * P)
    n_tiles = n_pad // P

    u16 = u.astype(np.float16)
    v16 = v.astype(np.float16)

    in_maps = []
    core_rows = []
    for c in range(N_CORES):
        rows = order[offsets[c] : offsets[c + 1]]
        core_rows.append(rows)
        cnt = len(rows)
        # u_t[p, h, n] = u[n, h*128+p]
        u_t = np.zeros((P, 2, n_pad), dtype=np.float16)
        ut = u16[rows].T.reshape(2, P, cnt)  # [h, p, n]
        u_t[:, :, :cnt] = ut.transpose(1, 0, 2)
        # v_p[p, t, e] = v[t*128+p, e]
        v_pad = np.zeros((n_pad, D), dtype=np.float16)
        v_pad[:cnt] = v16[rows]
        v_pc = np.ascontiguousarray(
            v_pad.reshape(n_tiles, P, D).transpose(1, 0, 2)
        )
        # pack per chunk: [u_h0 | u_h1 | v] each 4*ch fp16 per partition
        pk = np.empty((P, 4 * n_pad), dtype=np.float16)
        c0 = 0
        for ch in _chunk_sizes(n_pad):
            o = 4 * c0
            t0 = c0 // P
            pk[:, o : o + ch] = u_t[:, 0, c0 : c0 + ch]
            pk[:, o + ch : o + 2 * ch] = u_t[:, 1, c0 : c0 + ch]
            pk[:, o + 2 * ch : o + 4 * ch] = v_pc[
                :, t0 : t0 + ch // P, :
            ].reshape(P, 2 * ch)
            c0 += ch
        # w_p[p, h, e] = W[h*128+p, e]
        w16 = weights[c].astype(np.float16)
        w_pc = w16.reshape(2, P, D).transpose(1, 0, 2)
        in_maps.append(
            {
                "pk": pk,
                "w_p": np.ascontiguousarray(w_pc),
            }
        )
    return in_maps, core_rows, n_pad


def _run(u, v, weights, type_idx, trace=False):
    from concourse import bass_utils
    from concourse.bass_interp import get_hw_module

    n = np.asarray(u).shape[0]
    in_maps, core_rows, n_pad = _prep(u, v, weights, type_idx)
    n_tiles = n_pad // P

    nc = _get_program(n_pad)
    old_m = nc.m
    nc.m = get_hw_module(nc.m)
    try:
        res = bass_utils.run_bass_kernel_spmd(
            nc, in_maps, core_ids=list(range(N_CORES)), trace=trace
        )
    finally:
        nc.m = old_m

    final = np.empty((n,), dtype=np.float32)
    for c in range(N_CORES):
        arr = np.asarray(res.results[c]["out"]).reshape(P, n_tiles)
        per_row = arr.T.reshape(-1)[: len(core_rows[c])]
        final[core_rows[c]] = per_row
    return final, res


def kernel(**inputs) -> np.ndarray:
    out, _ = _run(
        inputs["u_hidden"],
        inputs["v_hidden"],
        inputs["weights"],
        inputs["type_idx"],
        trace=False,
    )
    return out



# revision 24
# speedup vs baseline: 1.0242x; 1.0242x over previous
"""Bilinear MoE-routing scores on 8 Trainium2 NeuronCores.

Problem: scores[n,k] = u[n,:] @ W_k @ v[n,:]; out[n] = sigmoid(scores[n, type_idx[n]]).
N=131072, D=256, K=8.

Sharding: rows grouped BY TYPE — core k gets exactly the rows with
type_idx == k, so each core runs one plain bilinear kernel against its own
W_k (8x less matmul work than data-parallel all-K). Host argsorts, pads each
group to a common n_pad, and scatters per-core results back to row order.

Precision: pure fp16 for u, v, AND W (validated offline on the exact
key=0 inputs: 6.0e-3 absmax output error vs the 2e-2 gate — fp32 PSUM
accumulation keeps products exact, only input rounding contributes).
This halves DMA bytes vs fp16-hi/lo + fp32-v: 1 KiB/row -> ~17 MB/core.

Device kernel (per core, SPMD):
  t[n,e] = sum_d uT[d,n] * W[d,e]   TensorE, fp16 x fp16 -> fp32 PSUM,
                                    2 matmuls per 128-row tile (~28 us)
  s[n]   = sum_e t[n,e] * v[n,e]    VectorE: one fused mul+cumsum custom op
                                    (MUL_CUMSUM_ANT, registered at import)
                                    per 8-tile PSUM supergroup; per-tile sums
                                    recovered as differences of the running
                                    sum at tile boundaries (ScalarE extracts
                                    the boundary column, VectorE diffs).
                                    ~37 us vs ~51 us for per-tile
                                    affine_mul_reduce (PSUM-source DVE ops run
                                    1x with a 120-cycle per-instruction
                                    penalty; the cumsum amortizes it 8x).
  out[n] = sigmoid(s[n])            ScalarE LUT

Measured (8 cores concurrent, paired-delta slope over 1024 on-device
iterations): ~47 us best / ~52-56 us under ambient HBM contention, vs
~52 us for a DMA-only ablation of the same traffic in the same window —
i.e. compute is fully hidden and the kernel sits at the HBM roofline
(16.9 MB/core; per-NC HBM limit is ~358 GB/s = 716 GB/s/stack / 2 NCs).
CHUNK=1536 with 6 DMA buffers edged out 1024/6 once the repeat loop was
unrolled (51.80 vs 52.06 med interleaved; pre-unroll, 1024/6 had beaten
2048/3 and 2048/6 — ambient load drifts minute-to-minute, so configs
must be compared interleaved in one process).  Sub-512-row leftover
chunks are folded into a neighboring mid chunk (a 128-row chunk means
256 B u-descriptor runs, below SDMA line rate).  u and v are packed
into ONE dram tensor laid out per chunk [u_h0 | u_h1 | v], so each
chunk is a single DMA (13/body instead of 26) with 4*ch contiguous
fp16 per partition, alternating between the two HWDGE rings per chunk:
-0.66 us interleaved (58.39 vs 59.05), consistent across quartiles —
same mechanism as the 1536-chunk win (fewer DMA instructions -> less
queue/semaphore pressure).  Packed-pool depth curve (in-flight DMAs):
5 -> 55.7, 6 -> 53.2, 8 -> 52.4 (optimum, adopted), 10 -> 52.8,
12 -> 52.7 us — packing halved the in-flight count, shifting the
optimum from the pre-pack 6+6 split pools to 8.  CHUNK=2048 in the
packed regime (16 KB descriptor runs/partition, 11 chunks/body) beat
1536 by a wide same-window margin (51.8 vs 58.9 med, whole
distribution shifted; pre-packing 2048 had LOST to 1536 — chunk-size
conclusions do not transfer across DMA-layout changes).  Removing the
fill/drain taper entirely is neutral-to-worse (59.3) — keep it.
Chunk DMAs alternate HWDGE rings via a global cross-body counter
(odd chunks/body would pin body boundaries to one ring); the output
drains joined the same counter and dropped from 4 blocks to 2: -1.9 us
interleaved (49.97 vs 51.88, whole distribution shifted) — at 4 blocks
on the sync ring, 512 tiny (~130 B) out-descriptors per body wedged
between the 16 KB input runs and stalled the SDMA engines on
descriptor overhead. UNROLL=4 invocations per For_i
iteration amortizes the ~2.2 us all-engine loop-reset barrier and lets
consecutive invocations' fill/drain overlap (parity-duplicated score
buffers break the WAR chain): 61.7 -> 54.1 (UNROLL=4) -> 52.7 us
(UNROLL=8) interleaved A/B medians; the unrolled dma_only floor is
~49.2 us, so ~3.5 us of cross-engine intrusion remains.  Duplicating
w_sb (PE weight reads alternate two SBUF copies per row-tile, halving
per-bank read pressure against concurrent DMA writes) bought a further
~0.5 us in two independent interleaved A/Bs.  Deeper DMA pools
(UBUFS/VBUFS 10) regress +2 us: more in-flight DMA worsens queueing.
G=4 with a 4-deep PSUM pool is neutral.
Rejected on HW measurement: end-taper chunks < 512 B/descriptor (+2 us:
sub-line-rate DMA on the latency-critical final bytes), offloading 1/3
of the reduction to a ScalarE evict+accum path (+18 us: real ScalarE
activation throughput is far below the (172+FD)/1.2GHz model), per-tile
affine_mul_reduce on fp16 SBUF (custom DVE ops have no 2x uops — all
reduces on DVE run 1x, so the fused PSUM-source scan is already
optimal). Baseline (fp16-hi/lo u + fp32 v, per-tile affine_mul_reduce):
~119 us.
"""

import math

import numpy as np

P = 128  # SBUF partitions
D = 256  # hidden dim
N_CORES = 8
CHUNK = 2048  # rows per DMA chunk (multiple of 128)
UBUFS = 8  # deep prefetch rides through transient HBM-contention dips
VBUFS = 6
G = 8  # row-tiles per PSUM super-tile ([128, G*256] f32 = G/2 banks)
SCAN_MODE = True  # fused mul+cumsum custom DVE op vs per-tile affine_mul_reduce
STAGGER = False  # staggered semaphore reset in the benchmark repeat loop (crashes HW)
UNROLL = 8  # kernel invocations per For_i iteration in the repeat loop.
# The For_i reset is an all-engine barrier (~2.2 us measured via the
# 'empty' ablation) and the fill/drain tail (~4 us) cannot cross it, so
# back-to-back invocations in one body pipeline naturally (score/output
# buffers are parity-duplicated to break WAR serialization) and the
# barrier+tail cost is amortized over UNROLL invocations — the slope
# then measures the true amortized streaming cost per invocation.

_PROGRAM_CACHE: dict = {}
_SCAN_OP = None


def _get_scan_op():
    """Register (once) and return the MUL_CUMSUM_ANT custom DVE op:
    out[p, k] = sum_{j<=k} in0[p, j] * in1[p, j]  (fp32 internal state).

    One fused 1x-rate pass replaces the per-tile affine_mul_reduce calls;
    per-instruction overhead is amortized over G row-tiles."""
    global _SCAN_OP
    if _SCAN_OP is not None:
        return _SCAN_OP
    import concourse.dve_ops as dve_ops
    from concourse.dve_spec import Spec, Src0, Src1, AluOp, scan, lower
    from concourse.dve_uop import DveOpSpec

    for o in dve_ops.OPS:
        if o.name == "MUL_CUMSUM_ANT":
            _SCAN_OP = o
            return o

    def _ref(in0, in1, s0, s1, imm2):
        return np.cumsum(
            in0.astype(np.float32) * np.asarray(in1, dtype=np.float32),
            axis=-1,
            dtype=np.float32,
        )

    spec = Spec(body=scan(AluOp.ADD, Src0 * Src1), reference=_ref)
    shas = {}
    for ver in ("v3", "v4"):
        uops = lower(spec, ver=ver)
        shas[ver] = DveOpSpec(
            name="MUL_CUMSUM_ANT", opcode=0, uops=uops, rd1_en=True
        ).sha(ver)
    op = dve_ops.DveOp("MUL_CUMSUM_ANT", spec, subdim=False, uops_sha=shas)
    dve_ops.OPS.append(op)
    dve_ops.CUSTOM_DVE_SPECS[op.name] = spec
    dve_ops._SUB_OPCODE_FOR_NAME[op.name] = (
        dve_ops._CUSTOM_DVE_ROW_BASE + len(dve_ops.OPS) - 1
    )
    _SCAN_OP = op
    return op


def _chunk_sizes(n_pad: int):
    """Small chunks at both ends for pipeline fill/drain, CHUNK in the middle.
    All sizes are multiples of 128; sum == n_pad."""
    rem = n_pad
    up = []
    for s in (512, 1024):
        if rem >= s + 1536 + CHUNK:
            up.append(s)
            rem -= s
    down = []
    for s in (1024, 512):
        if rem >= s + 512:
            down.append(s)
            rem -= s
    n_mid = rem // CHUNK
    leftover = rem - n_mid * CHUNK
    mids = [CHUNK] * n_mid
    if leftover and leftover < 512 and mids:
        # a sub-512-row chunk means sub-512B u descriptor runs (below SDMA
        # line rate) plus an extra DMA instruction — fold it into one mid
        # chunk instead
        mids[-1] += leftover
        leftover = 0
    sizes = up + mids + ([leftover] if leftover else []) + down
    assert sum(sizes) == n_pad and all(s % P == 0 for s in sizes)
    return sizes


def _build_program(n_pad: int, repeat: int = 1, mode: str = "full"):
    """Build + compile the SPMD Bass program for n_pad rows per core.

    repeat > 1 wraps the body in an on-device loop (benchmarking only).
    mode: 'full' | 'no_dve' | 'no_pe' | 'dma_only' (ablation benches)."""
    import contextlib

    import concourse.bass as bass  # noqa: F401
    import concourse.mybir as mybir
    import concourse.tile as tile
    from concourse import bacc

    do_pe = mode in ("full", "no_dve")
    do_dve = mode in ("full", "no_pe")
    do_dma = mode != "empty"

    f32 = mybir.dt.float32
    f16 = mybir.dt.float16
    n_tiles = n_pad // P
    assert n_pad % P == 0
    scan_op = _get_scan_op() if SCAN_MODE else None
    # number of supergroups (for the boundary-column buffer)
    n_groups = sum(
        len(range(0, ch // P, G)) for ch in _chunk_sizes(n_pad)
    )

    nc = bacc.Bacc(
        "TRN2", target_bir_lowering=False, debug=False, num_devices=N_CORES
    )
    # packed per-chunk: for chunk at rows [c0, c0+ch): u half0 (ch), u
    # half1 (ch), then v (2*ch) — one DMA per chunk, 4*ch contiguous fp16
    # per partition, issued on alternating HWDGE rings.
    pk = nc.dram_tensor("pk", [P, 4 * n_pad], f16, kind="ExternalInput").ap()
    # w pre-permuted: w_p[p, h, e] = W[h*128+p, e]
    w_p = nc.dram_tensor("w_p", [P, 2, D], f16, kind="ExternalInput").ap()
    out = nc.dram_tensor("out", [n_pad], f32, kind="ExternalOutput").ap()

    with tile.TileContext(nc) as tc:
        with (
            tc.tile_pool(name="singles", bufs=1) as singles,
            tc.tile_pool(name="pkpool", bufs=UBUFS) as pkpool,
            tc.tile_pool(name="ppool", bufs=max(1, 16 // G), space="PSUM") as ppool,
            tc.tile_pool(name="psingles", bufs=1, space="PSUM") as psingles,
            tc.tile_pool(name="spool", bufs=2) as spool,
        ):
            unroll = UNROLL if repeat > 1 else 1
            assert repeat == 1 or repeat % unroll == 0
            rep_ctx = (
                tc.For_i(
                    0,
                    repeat // unroll,
                    1,
                    hint_engines=(
                        mybir.EngineType.PE,
                        mybir.EngineType.DVE,
                        mybir.EngineType.Activation,
                    ),
                    staggered_reset=STAGGER,
                )
                if repeat > 1
                else contextlib.nullcontext()
            )

            # parity-duplicated so consecutive unrolled invocations overlap
            # without WAR serialization on the score/output buffers
            npar = min(unroll, 2)
            # s_buf[p, t] = score of padded row t*128+p
            s_bufs = [
                singles.tile([P, n_tiles], f32, tag=f"s{b}", name=f"s_buf{b}")
                for b in range(npar)
            ]
            sig_bufs = [
                singles.tile([P, n_tiles], f32, tag=f"sig{b}", name=f"sig_buf{b}")
                for b in range(npar)
            ]
            w_sbs = [
                singles.tile([P, 2, D], f16, tag=f"w{i}", name=f"w_sb{i}")
                for i in range(2)
            ]
            for _w in w_sbs:
                nc.scalar.dma_start(out=_w, in_=w_p)
            cum_bufs = []
            if SCAN_MODE:
                # cum[p, 0] = 0 permanently; the scan writes running sums of
                # the supergroup's products into cum[p, 1:1+g*D], so the sum
                # of tile j is cum[(j+1)*D] - cum[j*D] — one strided VectorE
                # subtract straight off the scan output, no ScalarE
                # boundary-copy hop.
                for b in range(2):
                    e = singles.tile([P, G * D + 1], f32, tag=f"cum{b}", name=f"cum{b}")
                    nc.vector.memset(e[:, 0:1], 0.0)
                    cum_bufs.append(e)
            static_ps = None
            if do_dve and not do_pe:
                # ablation: pre-zeroed PSUM tiles so the DVE reads allocated
                # data without any PE work inside the loop
                sps0 = psingles.tile([P, G, D], f32, tag="sps0")
                sps1 = psingles.tile([P, G, D], f32, tag="sps1")
                static_ps = [sps0, sps1]
                for sp in static_ps:
                    nc.vector.memset(sp, 0.0)

            out_pt = out.rearrange("(p t) -> p t", p=P)
            gci = [0]  # global chunk counter: strict HWDGE ring alternation
            # across body boundaries (odd chunks/body would otherwise put two
            # consecutive chunks on the same ring at every boundary)

            def emit_body(b):
                s_buf = s_bufs[b]
                sig_buf = sig_bufs[b]
                if not do_dma:
                    # empty-loop overhead probe: one tiny DVE op per body
                    nc.vector.memset(s_buf[:, 0:1], 0.0)
                c0 = 0
                gi = 0
                for ci, ch in enumerate(_chunk_sizes(n_pad) if do_dma else []):
                    cht = ch // P
                    t0 = c0 // P
                    pkt = pkpool.tile([P, 4 * ch], f16, tag="pk")
                    eng = nc.sync if gci[0] % 2 == 0 else nc.scalar
                    gci[0] += 1
                    eng.dma_start(out=pkt, in_=pk[:, 4 * c0 : 4 * c0 + 4 * ch])

                    for st in range(0, cht, G):
                        g = min(G, cht - st)
                        if do_pe:
                            ps = ppool.tile([P, g, D], f32, tag="ps")
                            for j in range(g):
                                w_sb = w_sbs[(st + j) % 2]
                                u0 = pkt[:, (st + j) * P : (st + j + 1) * P]
                                u1 = pkt[:, ch + (st + j) * P : ch + (st + j + 1) * P]
                                nc.tensor.matmul(
                                    ps[:, j, :], u0, w_sb[:, 0, :],
                                    start=True, stop=False,
                                )
                                nc.tensor.matmul(
                                    ps[:, j, :], u1, w_sb[:, 1, :],
                                    start=False, stop=True,
                                )
                        elif do_dve:
                            ps = static_ps[gi % 2][:, :g, :]
                        gt = t0 + st
                        if do_dve and SCAN_MODE:
                            cum = cum_bufs[gi % 2]
                            nc.vector._custom_dve(
                                scan_op,
                                out=cum[:, 1 : 1 + g * D],
                                in0=ps.rearrange("p g d -> p (g d)"),
                                in1=pkt[
                                    :,
                                    2 * ch + st * D : 2 * ch + (st + g) * D,
                                ],
                            )
                            hi = cum[:, 1 : 1 + g * D].rearrange(
                                "p (g d) -> p g d", d=D
                            )[:, :, D - 1]
                            lo = cum[:, 0 : g * D].rearrange(
                                "p (g d) -> p g d", d=D
                            )[:, :, 0]
                            nc.vector.tensor_tensor(
                                out=s_buf[:, gt : gt + g],
                                in0=hi,
                                in1=lo,
                                op=mybir.AluOpType.subtract,
                            )
                        elif do_dve:
                            scr = spool.tile([P, g, D], f32, tag="scr")
                            for j in range(g):
                                nc.vector.affine_mul_reduce(
                                    out=scr[:, j, :],
                                    accum_out=s_buf[:, gt + j : gt + j + 1],
                                    in0=ps[:, j, :],
                                    in1=pkt[:, 2 * ch + (st + j) * D : 2 * ch + (st + j + 1) * D],
                                    scale=1.0,
                                    bias=0.0,
                                )
                        gi += 1
                    c0 += ch

                # incremental sigmoid + output drain
                if do_dve:
                    n_blk = 2
                    bnd = [round(i * n_tiles / n_blk) for i in range(n_blk + 1)]
                    for b0, b1 in zip(bnd[:-1], bnd[1:]):
                        if b1 > b0:
                            nc.scalar.activation(
                                out=sig_buf[:, b0:b1],
                                in_=s_buf[:, b0:b1],
                                func=mybir.ActivationFunctionType.Sigmoid,
                            )
                            oeng = nc.sync if gci[0] % 2 == 0 else nc.scalar
                            gci[0] += 1
                            oeng.dma_start(
                                out=out_pt[:, b0:b1], in_=sig_buf[:, b0:b1]
                            )

            with rep_ctx:
                for b in range(unroll):
                    emit_body(b % npar)

    nc.compile()
    return nc


def _get_program(n_pad: int):
    if n_pad not in _PROGRAM_CACHE:
        _PROGRAM_CACHE[n_pad] = _build_program(n_pad)
    return _PROGRAM_CACHE[n_pad]


def _prep(u, v, weights, type_idx):
    """Group rows by type, pad, cast fp16, build per-core input maps."""
    u = np.ascontiguousarray(np.asarray(u, dtype=np.float32))
    v = np.ascontiguousarray(np.asarray(v, dtype=np.float32))
    weights = np.ascontiguousarray(np.asarray(weights, dtype=np.float32))
    ti = np.asarray(type_idx).astype(np.int64).ravel()

    n, d = u.shape
    k = weights.shape[0]
    assert d == D and k == N_CORES

    order = np.argsort(ti, kind="stable")
    counts = np.bincount(ti, minlength=k)
    offsets = np.concatenate(([0], np.cumsum(counts)))
    n_pad = max(P, int(math.ceil(counts.max() / P)) * P)
    n_tiles = n_pad // P

    u16 = u.astype(np.float16)
    v16 = v.astype(np.float16)

    in_maps = []
    core_rows = []
    for c in range(N_CORES):
        rows = order[offsets[c] : offsets[c + 1]]
        core_rows.append(rows)
        cnt = len(rows)
        # u_t[p, h, n] = u[n, h*128+p]
        u_t = np.zeros((P, 2, n_pad), dtype=np.float16)
        ut = u16[rows].T.reshape(2, P, cnt)  # [h, p, n]
        u_t[:, :, :cnt] = ut.transpose(1, 0, 2)
        # v_p[p, t, e] = v[t*128+p, e]
        v_pad = np.zeros((n_pad, D), dtype=np.float16)
        v_pad[:cnt] = v16[rows]
        v_pc = np.ascontiguousarray(
            v_pad.reshape(n_tiles, P, D).transpose(1, 0, 2)
        )
        # pack per chunk: [u_h0 | u_h1 | v] each 4*ch fp16 per partition
        pk = np.empty((P, 4 * n_pad), dtype=np.float16)
        c0 = 0
        for ch in _chunk_sizes(n_pad):
            o = 4 * c0
            t0 = c0 // P
            pk[:, o : o + ch] = u_t[:, 0, c0 : c0 + ch]
            pk[:, o + ch : o + 2 * ch] = u_t[:, 1, c0 : c0 + ch]
            pk[:, o + 2 * ch : o + 4 * ch] = v_pc[
                :, t0 : t0 + ch // P, :
            ].reshape(P, 2 * ch)
            c0 += ch
        # w_p[p, h, e] = W[h*128+p, e]
        w16 = weights[c].astype(np.float16)
        w_pc = w16.reshape(2, P, D).transpose(1, 0, 2)
        in_maps.append(
            {
                "pk": pk,
                "w_p": np.ascontiguousarray(w_pc),
            }
        )
    return in_maps, core_rows, n_pad


def _run(u, v, weights, type_idx, trace=False):
    from concourse import bass_utils
    from concourse.bass_interp import get_hw_module

    n = np.asarray(u).shape[0]
    in_maps, core_rows, n_pad = _prep(u, v, weights, type_idx)
    n_tiles = n_pad // P

    nc = _get_program(n_pad)
    old_m = nc.m
    nc.m = get_hw_module(nc.m)
    try:
        res = bass_utils.run_bass_kernel_spmd(
            nc, in_maps, core_ids=list(range(N_CORES)), trace=trace
        )
    finally:
        nc.m = old_m

    final = np.empty((n,), dtype=np.float32)
    for c in range(N_CORES):
        arr = np.asarray(res.results[c]["out"]).reshape(P, n_tiles)
        per_row = arr.T.reshape(-1)[: len(core_rows[c])]
        final[core_rows[c]] = per_row
    return final, res


def kernel(**inputs) -> np.ndarray:
    out, _ = _run(
        inputs["u_hidden"],
        inputs["v_hidden"],
        inputs["weights"],
        inputs["type_idx"],
        trace=False,
    )
    return out

